# revision 16
# baseline (speedup 1.0000x reference)
"""Trainium2 Bass kernel for nn_EnhancedStateEncoder (6-layer dense transformer).

Strategy: data-parallel over batch across 8 NeuronCores (2 batches/core).
 - Embedding + sinusoidal pos-emb folded on host (cheap gather).
 - exp(alibi_bias) [H,S,S] precomputed host-side (input-independent, cached),
   stored transposed as bf16, resident in SBUF. Softmax numerator is
   exp(SCALE*qk) * exp(bias): the exp runs on the Scalar engine straight out
   of PSUM and the static exp(bias) factor is a DVE bf16 multiply (2x mode),
   keeping the bias add off the TensorEngine entirely.
 - All matmuls in bf16 (1 cycle/row vs 4 for fp32). QK^T contracts over only
   HD=32 dims, so adjacent-in-time matmuls for two heads are placed on
   different 32-row PE bands (tile_position) and run concurrently.
 - Attention computed in S^T layout [j(part), i(free)]; a ones-column
   appended to V yields softmax denominators for free (M=33 stationary).
 - LayerNorm via bn_stats/bn_aggr; rsqrt computed as exp(-0.5*ln(var+eps))
   to stay inside the natural_log_exp activation-table set.
 - LN2's affine is folded into the MLP's first matmul on the host.
"""

import math
import os
from contextlib import ExitStack

import numpy as np
import ml_dtypes

import concourse.bass as bass
import concourse.mybir as mybir
import concourse.tile as tile
from concourse.bass_utils import run_bass_kernel_spmd
from concourse.masks import make_identity

F32 = mybir.dt.float32
BF16 = mybir.dt.bfloat16

B, S, D, H, HD, L, H2 = 16, 1024, 256, 8, 32, 6, 1024
NC = 8            # cores
BL = B // NC      # batches per core = 2
T = BL * S        # tokens per core = 2048
NCH = T // 128    # 128-token chunks per core = 16
SCALE = 1.0 / math.sqrt(HD)
LN_EPS = 1e-5
GRID = 32

_cache = {}


def _alibi_ebT():
    """ebT[p, h, jc, i] = exp(bias[h, i, jc*128+p]), bf16."""
    if "ebT" in _cache:
        return _cache["ebT"]
    xs, ys = np.meshgrid(np.arange(GRID), np.arange(GRID), indexing="ij")
    xf = xs.reshape(-1).astype(np.float32)
    yf = ys.reshape(-1).astype(np.float32)
    dist = np.abs(xf[:, None] - xf[None, :]) + np.abs(yf[:, None] - yf[None, :])
    i = np.arange(H, dtype=np.float32)
    sl = -(2.0 ** (-(1.0 + i)))
    sr = -(2.0 ** (-(0.5 + i)))
    out = np.empty((128, H, S // 128, S), dtype=ml_dtypes.bfloat16)
    triu = np.triu(np.ones((S, S), np.bool_))  # j >= i
    for h in range(H):
        b = np.exp(np.where(triu, sr[h] * dist, sl[h] * dist))  # [i, j]
        bT = np.ascontiguousarray(b.T)  # [j, i]
        out[:, h] = bT.reshape(S // 128, 128, S).transpose(1, 0, 2)
    _cache["ebT"] = out
    return out


def _pos_table():
    if "pos" in _cache:
        return _cache["pos"]
    inv_freq = 1.0 / (10000.0 ** (np.arange(0, D, 2, dtype=np.float32) / D))
    t = np.arange(S, dtype=np.float32)
    sinusoid = t[:, None] * inv_freq[None, :]
    _cache["pos"] = np.concatenate(
        [np.sin(sinusoid), np.cos(sinusoid)], axis=-1
    ).astype(np.float32)
    return _cache["pos"]


def _build_bass():
    if "nc" in _cache:
        return _cache["nc"]
    nc = bass.Bass()
    io = {}
    io["x0"] = nc.dram_tensor("x0", [128, NCH, D], F32, kind="ExternalInput")
    io["ebT"] = nc.dram_tensor("ebT", [128, H, S // 128, S], BF16, kind="ExternalInput")
    io["w1h"] = nc.dram_tensor("w1h", [L, 128, D // 128, H2], BF16, kind="ExternalInput")
    io["b1h"] = nc.dram_tensor("b1h", [L, 128, H2 // 128], F32, kind="ExternalInput")
    io["w2h"] = nc.dram_tensor("w2h", [L, 128, H2 // 128, D], BF16, kind="ExternalInput")
    io["b2h"] = nc.dram_tensor("b2h", [L, 1, D], BF16, kind="ExternalInput")
    io["ln1w"] = nc.dram_tensor("ln1w", [L, 128, D], BF16, kind="ExternalInput")
    io["ln1b"] = nc.dram_tensor("ln1b", [L, 128, D], BF16, kind="ExternalInput")
    io["lnfw"] = nc.dram_tensor("lnfw", [128, D], BF16, kind="ExternalInput")
    io["lnfb"] = nc.dram_tensor("lnfb", [128, D], BF16, kind="ExternalInput")
    y = nc.dram_tensor("y", [128, NCH, D], F32, kind="ExternalOutput")

    with tile.TileContext(nc) as tc, ExitStack() as ctx:
        _emit(ctx, tc, io, y)

    _split_multi_waits(nc)
    _cache["nc"] = nc
    return nc


def _split_multi_waits(nc):
    """walrus codegen on this image only supports ONE sync-wait per TPB
    engine-instruction descriptor. Move excess waits onto sequencer NoOps
    inserted immediately before the instruction (same engine queue)."""
    nsplit = 0
    skip = ("InstNoOp", "InstEventSemaphore")
    for func in nc.m.functions:
        for bb in func.blocks:
            insts = list(bb.instructions)
            out = []
            for inst in insts:
                si = inst.sync_info
                if (si is not None and si.on_wait and len(si.on_wait) > 1
                        and type(inst).__name__ not in skip):
                    for w in list(si.on_wait[:-1]):
                        nop = mybir.InstNoOp(
                            name=f"WSPLIT-{nsplit}", ins=[], outs=[])
                        nop.engine = inst.engine
                        nop.sync_info = mybir.SyncInfo(
                            on_wait=[w], on_update=[])
                        out.append(nop)
                        nsplit += 1
                    si.on_wait = [si.on_wait[-1]]
                out.append(inst)
            if nsplit:
                bb.instructions = out
    return nsplit


def _emit(ctx, tc, io, y):
    nc = tc.nc
    singles = ctx.enter_context(tc.tile_pool(name="singles", bufs=1))
    lnp = ctx.enter_context(tc.tile_pool(name="lnp", bufs=1))
    wp = ctx.enter_context(tc.tile_pool(name="wp", bufs=1))
    xp = ctx.enter_context(tc.tile_pool(name="xp", bufs=2))
    sp = ctx.enter_context(tc.tile_pool(name="sp", bufs=4))
    ep = ctx.enter_context(tc.tile_pool(name="ep", bufs=8))
    otp = ctx.enter_context(tc.tile_pool(name="otp", bufs=2))
    tmp_p = ctx.enter_context(tc.tile_pool(name="tmp", bufs=2))
    # PSUM: ps pool (2 bufs x [128,1024]f32 = 4 banks) for QK scores, LN
    # transposes, PV-out transposes and MLP tiles; po pool (2 bufs x
    # [33,1024]f32 = 4 banks) for PV accumulation.
    ps_p = ctx.enter_context(tc.tile_pool(name="ps", bufs=2, space="PSUM"))
    po_p = ctx.enter_context(tc.tile_pool(name="po", bufs=2, space="PSUM"))

    # ---- resident tensors ----
    x_sb = singles.tile([128, NCH, D], F32)
    nc.sync.dma_start(out=x_sb, in_=io["x0"][:])
    eb_sb = singles.tile([128, H, S // 128, S], BF16)
    nc.sync.dma_start(out=eb_sb, in_=io["ebT"][:])
    id_f32 = singles.tile([128, 128], F32)
    make_identity(nc, id_f32)
    id_bf16 = singles.tile([128, 128], BF16)
    nc.gpsimd.tensor_copy(out=id_bf16, in_=id_f32)
    ones_col = singles.tile([1, 128], BF16)
    nc.vector.memset(ones_col, 1.0)
    eps_t = singles.tile([128, 1], F32)
    nc.vector.memset(eps_t, LN_EPS)
    absorb_scratch = singles.tile([128, 16], F32)
    absorb_n = [0]

    def absorb(ap):
        # DVE wait absorber: DVE-struct instructions support only one sync
        # wait on this codegen, so soak the DMA-completion wait into a copy.
        # Disjoint dest columns so absorbers carry no WAW dep on each other.
        k = absorb_n[0] % 16
        absorb_n[0] += 1
        nc.vector.tensor_copy(out=absorb_scratch[:, k:k + 1],
                              in_=ap[0:128, 0:1])
    v_aug = singles.tile([128, NCH, H, 34], BF16)
    nc.vector.memset(v_aug, 1.0)
    # xnT: [128 (4 heads x 32 dims), half, T] transposed LN'd activations
    xnT = singles.tile([128, 2, T], BF16)
    hT = singles.tile([128, 8, 512], BF16)

    def ln_stats(c, mv):
        st = sp.tile([128, 6], F32, tag="st")
        nc.vector.bn_stats(out=st, in_=x_sb[:, c, :])
        nc.vector.bn_aggr(out=mv[:, c, :], in_=st)

    def ln_rsqrt(mv, rs):
        nc.scalar.activation(
            out=rs, in_=mv[:, :, 1],
            func=mybir.ActivationFunctionType.Ln, bias=eps_t, scale=1.0,
        )
        nc.scalar.activation(
            out=rs, in_=rs,
            func=mybir.ActivationFunctionType.Exp, scale=-0.5,
        )

    def ln_norm(c, mv, rs, affine):
        """normalized bf16 chunk of x_sb[:, c, :]"""
        xn = xp.tile([128, D], BF16, tag="xn")
        if affine is not None:
            w_sb, b_sb = affine
            xf = xp.tile([128, D], F32, tag="xf")
            nc.vector.tensor_scalar(
                out=xf, in0=x_sb[:, c, :],
                scalar1=mv[:, c, 0:1], scalar2=rs[:, c:c + 1],
                op0=mybir.AluOpType.subtract, op1=mybir.AluOpType.mult,
            )
            nc.vector.tensor_mul(out=xf, in0=xf, in1=w_sb)
            nc.vector.tensor_add(out=xn, in0=xf, in1=b_sb)
        else:
            nc.vector.tensor_scalar(
                out=xn, in0=x_sb[:, c, :],
                scalar1=mv[:, c, 0:1], scalar2=rs[:, c:c + 1],
                op0=mybir.AluOpType.subtract, op1=mybir.AluOpType.mult,
            )
        return xn

    def transpose_to(xn, c):
        # one PE transpose per 128-col half into a single 1-bank PSUM tile,
        # then one DVE copy into both xnT halves.
        pt = ps_p.tile([128, 2, 128], BF16, tag="ps")
        for half in range(2):
            nc.tensor.transpose(
                pt[:, half, :], xn[:, half * 128:(half + 1) * 128], id_bf16
            )
        nc.vector.tensor_copy(
            out=xnT[:, :, c * 128:(c + 1) * 128], in_=pt
        )

    mvA = sp.tile([128, NCH, 2], F32, tag="mvA")
    for c in range(NCH):
        ln_stats(c, mvA)

    for l in range(L):
        # per-layer params
        ln1w_sb = lnp.tile([128, D], BF16, tag="ln1w")
        nc.sync.dma_start(out=ln1w_sb, in_=io["ln1w"][l])
        absorb(ln1w_sb)
        ln1b_sb = lnp.tile([128, D], BF16, tag="ln1b")
        nc.sync.dma_start(out=ln1b_sb, in_=io["ln1b"][l])
        absorb(ln1b_sb)
        w1_sb = wp.tile([128, D // 128, H2], BF16, tag="w1")
        nc.sync.dma_start(out=w1_sb, in_=io["w1h"][l])
        b1_sb = wp.tile([128, H2 // 128], F32, tag="b1")
        nc.sync.dma_start(out=b1_sb, in_=io["b1h"][l])
        w2_sb = wp.tile([128, H2 // 128, D], BF16, tag="w2")
        nc.sync.dma_start(out=w2_sb, in_=io["w2h"][l])
        b2_sb = wp.tile([1, D], BF16, tag="b2")
        nc.sync.dma_start(out=b2_sb, in_=io["b2h"][l])

        # ---- phase A: LN1 (stats in mvA from previous phase D) ----
        rsA = sp.tile([128, NCH], F32, tag="rsA")
        ln_rsqrt(mvA, rsA)
        for c in range(NCH):
            xn = ln_norm(c, mvA, rsA, (ln1w_sb, ln1b_sb))
            nc.gpsimd.tensor_copy(
                out=v_aug[:, c, :, 0:HD],
                in_=xn.rearrange("p (h d) -> p h d", h=H),
            )
            transpose_to(xn, c)

        # ---- phase B: attention, software-pipelined across head slots ----
        def qk_exp_ebm(b, h, jc):
            hh, hp = h // 4, (h % 4) * HD
            ps = ps_p.tile([128, S], F32, tag="ps")
            ktile = xnT[hp:hp + HD, hh,
                        b * S + jc * 128: b * S + (jc + 1) * 128]
            for it in range(2):
                qtile = xnT[hp:hp + HD, hh,
                            b * S + it * 512: b * S + (it + 1) * 512]
                nc.tensor.matmul(
                    ps[:, it * 512:(it + 1) * 512],
                    lhsT=ktile, rhs=qtile, start=True, stop=True,
                    tile_position=(hp, 0),
                )
            et = ep.tile([128, S], BF16, tag="et")
            nc.scalar.activation(
                out=et, in_=ps,
                func=mybir.ActivationFunctionType.Exp, scale=SCALE,
            )
            nc.vector.tensor_mul(out=et, in0=et, in1=eb_sb[:, h, jc, :])
            return et

        def pv(b, h, jc, po, etb):
            for it in range(2):
                nc.tensor.matmul(
                    po[:, it * 512:(it + 1) * 512],
                    lhsT=v_aug[:, b * 8 + jc, h, 0:33],
                    rhs=etb[:, it * 512:(it + 1) * 512],
                    start=(jc == 0), stop=(jc == S // 128 - 1),
                )

        def po_post(b, h, po):
            # denominator division + residual add, batched per (b, h)
            ot = otp.tile([33, S], BF16, tag="ot")
            nc.vector.tensor_copy(out=ot, in_=po)
            ptt = po_p.tile([128, 8, 34], BF16, tag="po")
            for ic in range(S // 128):
                nc.tensor.transpose(
                    ptt[:, ic, 0:33], ot[:, ic * 128:(ic + 1) * 128],
                    id_bf16[0:33, 0:33],
                )
            rt = sp.tile([128, 8, 1], F32, tag="rt")
            nc.vector.reciprocal(out=rt, in_=ptt[:, :, 32:33])
            tmp = tmp_p.tile([128, 8, HD], BF16, tag="tmp")
            nc.vector.tensor_tensor(
                out=tmp, in0=ptt[:, :, 0:HD],
                in1=rt[:, :, :].broadcast_to([128, 8, HD]),
                op=mybir.AluOpType.mult,
            )
            xs = x_sb[:, b * 8:(b + 1) * 8, h * HD:(h + 1) * HD]
            nc.vector.tensor_add(out=xs, in0=xs, in1=tmp)

        mvC = sp.tile([128, NCH, 2], F32, tag="mvC")
        pend = []
        po_cur = {}

        def drain_one():
            b0, h0, j0, e0 = pend.pop(0)
            if j0 == 0:
                po_t = po_p.tile([33, S], F32, tag="po")
                po_cur[(b0, h0)] = po_t
            pv(b0, h0, j0, po_cur[(b0, h0)], e0)
            if j0 == S // 128 - 1:
                po_post(b0, h0, po_cur.pop((b0, h0)))
                if h0 == H - 1:
                    # this batch's residual stream is final: LN2 stats can
                    # ride the DVE queue under the remaining attention slots
                    for c in range(b0 * 8, b0 * 8 + 8):
                        ln_stats(c, mvC)

        for b in range(BL):
            for h in range(H):
                for jc in range(S // 128):
                    etb = qk_exp_ebm(b, h, jc)
                    pend.append((b, h, jc, etb))
                    if len(pend) > 5:
                        drain_one()
        while pend:
            drain_one()

        # ---- phase C (LN2, affine folded into w1) fused with phase D ----
        rsC = sp.tile([128, NCH], F32, tag="rsC")
        ln_rsqrt(mvC, rsC)
        mvA = sp.tile([128, NCH, 2], F32, tag="mvA")
        for tt in range(T // 512):
            for c in range(4 * tt, 4 * tt + 4):
                transpose_to(ln_norm(c, mvC, rsC, None), c)
            for hbp in range(4):  # pairs of h2-blocks
                pm = ps_p.tile([128, S], F32, tag="ps")
                for sub in range(2):
                    hb = hbp * 2 + sub
                    for k in range(D // 128):
                        nc.tensor.matmul(
                            pm[:, sub * 512:(sub + 1) * 512],
                            lhsT=w1_sb[:, k, hb * 128:(hb + 1) * 128],
                            rhs=xnT[:, k, tt * 512:(tt + 1) * 512],
                            start=(k == 0), stop=(k == D // 128 - 1),
                        )
                for sub in range(2):
                    hb = hbp * 2 + sub
                    nc.scalar.activation(
                        out=hT[:, hb, :],
                        in_=pm[:, sub * 512:(sub + 1) * 512],
                        func=mybir.ActivationFunctionType.Gelu,
                        bias=b1_sb[:, hb:hb + 1],
                    )
            for t2 in range(4):
                pm2 = ps_p.tile([128, D], F32, tag="ps")
                for hb in range(H2 // 128):
                    nc.tensor.matmul(
                        pm2,
                        lhsT=hT[:, hb, t2 * 128:(t2 + 1) * 128],
                        rhs=w2_sb[:, hb, :],
                        start=(hb == 0), stop=False,
                    )
                nc.tensor.matmul(
                    pm2, lhsT=ones_col, rhs=b2_sb, start=False, stop=True
                )
                c = tt * 4 + t2
                nc.vector.tensor_add(
                    out=x_sb[:, c, :], in0=x_sb[:, c, :], in1=pm2
                )
                # stats for the next LN (LN1 of l+1, or the final LN)
                ln_stats(c, mvA)

    # ---- final LN (in place, f32) ----
    lnfw_sb = lnp.tile([128, D], BF16, tag="ln1w")
    nc.sync.dma_start(out=lnfw_sb, in_=io["lnfw"][:])
    absorb(lnfw_sb)
    lnfb_sb = lnp.tile([128, D], BF16, tag="ln1b")
    nc.sync.dma_start(out=lnfb_sb, in_=io["lnfb"][:])
    absorb(lnfb_sb)

    rsF = sp.tile([128, NCH], F32, tag="rsA")
    ln_rsqrt(mvA, rsF)
    for c in range(NCH):
        xc = x_sb[:, c, :]
        nc.vector.tensor_scalar(
            out=xc, in0=xc,
            scalar1=mvA[:, c, 0:1], scalar2=rsF[:, c:c + 1],
            op0=mybir.AluOpType.subtract, op1=mybir.AluOpType.mult,
        )
        nc.vector.tensor_mul(out=xc, in0=xc, in1=lnfw_sb)
        nc.vector.tensor_add(out=xc, in0=xc, in1=lnfb_sb)
    nc.sync.dma_start(out=y[:], in_=x_sb)


def _install_ntff_hook():
    """Wire antenv.axon_hooks NTFF profiling via libaxon ctypes (dev only)."""
    if _cache.get("hook_done"):
        return
    _cache["hook_done"] = True
    try:
        import types
        import sys
        try:
            from antenv.axon_hooks import set_axon_ntff_profile_hook  # noqa
        except ImportError:
            import antenv
            mod = types.ModuleType("antenv.axon_hooks")
            holder = [None]
            mod.set_axon_ntff_profile_hook = lambda h: holder.__setitem__(0, h)
            mod.get_axon_ntff_profile_hook = lambda: holder[0]
            sys.modules["antenv.axon_hooks"] = mod
            antenv.axon_hooks = mod
            from trn_agent_boot.trn_boot import _ntff_profile_via_ctypes
            mod.set_axon_ntff_profile_hook(
                _ntff_profile_via_ctypes("/opt/axon/libaxon_pjrt.so"))
    except Exception as e:  # fail-soft: tracing degrades, run still works
        print("ntff hook install failed:", e)


def kernel(tokens, pos_ids, emb_table, input_weight, position_weight,
           ln1_w, ln1_b, ln2_w, ln2_b, w1, b1, w2, b2, lnf_w, lnf_b):
    tokens = np.asarray(tokens)
    pos_ids = np.asarray(pos_ids)
    emb_table = np.asarray(emb_table, dtype=np.float32)
    x0 = (np.float32(np.asarray(input_weight).reshape(-1)[0])
          * emb_table[tokens]
          + np.float32(np.asarray(position_weight).reshape(-1)[0])
          * _pos_table()[np.asarray(pos_ids)][None]).astype(np.float32)

    w1 = np.asarray(w1, np.float32)
    b1 = np.asarray(b1, np.float32)
    w2 = np.asarray(w2, np.float32)
    b2 = np.asarray(b2, np.float32)
    ln2_w = np.asarray(ln2_w, np.float32)
    ln2_b = np.asarray(ln2_b, np.float32)
    # fold LN2 affine into MLP weights
    w1eff = ln2_w[:, :, None] * w1                     # [L, D, H2]
    b1eff = b1 + np.einsum("ld,ldh->lh", ln2_b, w1)    # [L, H2]
    w1h = np.ascontiguousarray(
        w1eff.reshape(L, D // 128, 128, H2).transpose(0, 2, 1, 3)
    ).astype(ml_dtypes.bfloat16)
    w2h = np.ascontiguousarray(
        w2.reshape(L, H2 // 128, 128, D).transpose(0, 2, 1, 3)
    ).astype(ml_dtypes.bfloat16)

    nc = _build_bass()
    base = {
        "ebT": _alibi_ebT(),
        "w1h": w1h,
        "b1h": np.ascontiguousarray(
            b1eff.reshape(L, H2 // 128, 128).transpose(0, 2, 1)),
        "w2h": w2h,
        "b2h": np.ascontiguousarray(
            np.asarray(b2)[:, None, :]).astype(ml_dtypes.bfloat16),
        "ln1w": np.ascontiguousarray(np.broadcast_to(
            np.asarray(ln1_w)[:, None, :], (L, 128, D))
        ).astype(ml_dtypes.bfloat16),
        "ln1b": np.ascontiguousarray(np.broadcast_to(
            np.asarray(ln1_b)[:, None, :], (L, 128, D))
        ).astype(ml_dtypes.bfloat16),
        "lnfw": np.ascontiguousarray(np.broadcast_to(
            np.asarray(lnf_w)[None, :], (128, D))
        ).astype(ml_dtypes.bfloat16),
        "lnfb": np.ascontiguousarray(np.broadcast_to(
            np.asarray(lnf_b)[None, :], (128, D))
        ).astype(ml_dtypes.bfloat16),
    }
    in_maps = []
    for core in range(NC):
        xc = x0[core * BL:(core + 1) * BL].reshape(T, D)
        xh = np.ascontiguousarray(
            xc.reshape(NCH, 128, D).transpose(1, 0, 2))
        m = dict(base)
        m["x0"] = xh
        in_maps.append(m)

    trace = os.environ.get("KERNEL_TRACE", "0") == "1"
    if trace:
        _install_ntff_hook()
    res = run_bass_kernel_spmd(
        nc, in_maps, core_ids=list(range(NC)), trace=trace,
        trace_cores=[0] if trace else None,
    )
    if trace and res.exec_time_ns is not None:
        print(f"HW exec time: {res.exec_time_ns} ns")
        if res.instructions_and_trace is not None:
            print("trace:", res.instructions_and_trace[1])

    out = np.empty((B, S, D), np.float32)
    for core in range(NC):
        yh = res.results[core]["y"]  # [128, NCH, D]
        yc = yh.transpose(1, 0, 2).reshape(BL, S, D)
        out[core * BL:(core + 1) * BL] = yc
    return out


# revision 17
# speedup vs baseline: 1.1857x; 1.1857x over previous
"""Trainium2 Bass kernel for nn_EnhancedStateEncoder (6-layer dense transformer).

Strategy: data-parallel over batch across 8 NeuronCores (2 batches/core).
 - Embedding + sinusoidal pos-emb folded on host (cheap gather).
 - exp(alibi_bias) [H,S,S] precomputed host-side (input-independent, cached),
   stored transposed as bf16, resident in SBUF. Softmax numerator is
   exp(SCALE*qk) * exp(bias): the exp runs on the Scalar engine straight out
   of PSUM and the static exp(bias) factor is a DVE bf16 multiply (2x mode),
   keeping the bias add off the TensorEngine entirely.
 - All matmuls in bf16 (1 cycle/row vs 4 for fp32). QK^T contracts over only
   HD=32 dims, so adjacent-in-time matmuls for two heads are placed on
   different 32-row PE bands (tile_position) and run concurrently.
 - Attention computed in S^T layout [j(part), i(free)]; a ones-column
   appended to V yields softmax denominators for free (M=33 stationary).
 - LayerNorm via bn_stats/bn_aggr; rsqrt computed as exp(-0.5*ln(var+eps))
   to stay inside the natural_log_exp activation-table set.
 - LN2's affine is folded into the MLP's first matmul on the host.
"""

import math
import os
from contextlib import ExitStack

import numpy as np
import ml_dtypes

import concourse.bass as bass
import concourse.mybir as mybir
import concourse.tile as tile
from concourse.bass_utils import run_bass_kernel_spmd
from concourse.masks import make_identity

F32 = mybir.dt.float32
BF16 = mybir.dt.bfloat16

B, S, D, H, HD, L, H2 = 16, 1024, 256, 8, 32, 6, 1024
NC = 8            # cores
BL = B // NC      # batches per core = 2
T = BL * S        # tokens per core = 2048
NCH = T // 128    # 128-token chunks per core = 16
SCALE = 1.0 / math.sqrt(HD)
LN_EPS = 1e-5
GRID = 32

_cache = {}


def _alibi_ebT():
    """ebT[p, h, jc, i] = exp(bias[h, i, jc*128+p]), bf16."""
    if "ebT" in _cache:
        return _cache["ebT"]
    xs, ys = np.meshgrid(np.arange(GRID), np.arange(GRID), indexing="ij")
    xf = xs.reshape(-1).astype(np.float32)
    yf = ys.reshape(-1).astype(np.float32)
    dist = np.abs(xf[:, None] - xf[None, :]) + np.abs(yf[:, None] - yf[None, :])
    i = np.arange(H, dtype=np.float32)
    sl = -(2.0 ** (-(1.0 + i)))
    sr = -(2.0 ** (-(0.5 + i)))
    out = np.empty((128, H, S // 128, S), dtype=ml_dtypes.bfloat16)
    triu = np.triu(np.ones((S, S), np.bool_))  # j >= i
    for h in range(H):
        b = np.exp(np.where(triu, sr[h] * dist, sl[h] * dist))  # [i, j]
        bT = np.ascontiguousarray(b.T)  # [j, i]
        out[:, h] = bT.reshape(S // 128, 128, S).transpose(1, 0, 2)
    _cache["ebT"] = out
    return out


def _pos_table():
    if "pos" in _cache:
        return _cache["pos"]
    inv_freq = 1.0 / (10000.0 ** (np.arange(0, D, 2, dtype=np.float32) / D))
    t = np.arange(S, dtype=np.float32)
    sinusoid = t[:, None] * inv_freq[None, :]
    _cache["pos"] = np.concatenate(
        [np.sin(sinusoid), np.cos(sinusoid)], axis=-1
    ).astype(np.float32)
    return _cache["pos"]


def _build_bass():
    if "nc" in _cache:
        return _cache["nc"]
    nc = bass.Bass()
    io = {}
    io["x0"] = nc.dram_tensor("x0", [128, NCH, D], F32, kind="ExternalInput")
    io["ebT"] = nc.dram_tensor("ebT", [128, H, S // 128, S], BF16, kind="ExternalInput")
    io["w1h"] = nc.dram_tensor("w1h", [L, 128, D // 128, H2], BF16, kind="ExternalInput")
    io["b1h"] = nc.dram_tensor("b1h", [L, 128, H2 // 128], F32, kind="ExternalInput")
    io["w2h"] = nc.dram_tensor("w2h", [L, 128, H2 // 128, D], BF16, kind="ExternalInput")
    io["b2h"] = nc.dram_tensor("b2h", [L, 1, D], BF16, kind="ExternalInput")
    io["ln1w"] = nc.dram_tensor("ln1w", [L, 128, D], BF16, kind="ExternalInput")
    io["ln1b"] = nc.dram_tensor("ln1b", [L, 128, D], BF16, kind="ExternalInput")
    io["lnfw"] = nc.dram_tensor("lnfw", [128, D], BF16, kind="ExternalInput")
    io["lnfb"] = nc.dram_tensor("lnfb", [128, D], BF16, kind="ExternalInput")
    y = nc.dram_tensor("y", [128, NCH, D], F32, kind="ExternalOutput")

    with tile.TileContext(nc) as tc, ExitStack() as ctx:
        _emit(ctx, tc, io, y)

    _split_multi_waits(nc)
    _cache["nc"] = nc
    return nc


def _split_multi_waits(nc):
    """walrus codegen on this image only supports ONE sync-wait per TPB
    engine-instruction descriptor. Move excess waits onto sequencer NoOps
    inserted immediately before the instruction (same engine queue)."""
    nsplit = 0
    skip = ("InstNoOp", "InstEventSemaphore")
    for func in nc.m.functions:
        for bb in func.blocks:
            insts = list(bb.instructions)
            out = []
            for inst in insts:
                si = inst.sync_info
                if (si is not None and si.on_wait and len(si.on_wait) > 1
                        and type(inst).__name__ not in skip):
                    for w in list(si.on_wait[:-1]):
                        nop = mybir.InstNoOp(
                            name=f"WSPLIT-{nsplit}", ins=[], outs=[])
                        nop.engine = inst.engine
                        nop.sync_info = mybir.SyncInfo(
                            on_wait=[w], on_update=[])
                        out.append(nop)
                        nsplit += 1
                    si.on_wait = [si.on_wait[-1]]
                out.append(inst)
            if nsplit:
                bb.instructions = out
    return nsplit


def _emit(ctx, tc, io, y):
    nc = tc.nc
    singles = ctx.enter_context(tc.tile_pool(name="singles", bufs=1))
    lnp = ctx.enter_context(tc.tile_pool(name="lnp", bufs=1))
    wp = ctx.enter_context(tc.tile_pool(name="wp", bufs=1))
    xp = ctx.enter_context(tc.tile_pool(name="xp", bufs=2))
    sp = ctx.enter_context(tc.tile_pool(name="sp", bufs=4))
    ep = ctx.enter_context(tc.tile_pool(name="ep", bufs=8))
    otp = ctx.enter_context(tc.tile_pool(name="otp", bufs=2))
    tmp_p = ctx.enter_context(tc.tile_pool(name="tmp", bufs=2))
    # PSUM: ps pool (2 bufs x [128,1024]f32 = 4 banks) for QK scores, LN
    # transposes, PV-out transposes and MLP tiles; po pool (2 bufs x
    # [33,1024]f32 = 4 banks) for PV accumulation.
    ps_p = ctx.enter_context(tc.tile_pool(name="ps", bufs=3, space="PSUM"))
    po_p = ctx.enter_context(tc.tile_pool(name="po", bufs=1, space="PSUM"))

    # ---- resident tensors ----
    x_sb = singles.tile([128, NCH, D], F32)
    nc.sync.dma_start(out=x_sb, in_=io["x0"][:])
    eb_sb = singles.tile([128, H, S // 128, S], BF16)
    nc.sync.dma_start(out=eb_sb, in_=io["ebT"][:])
    id_f32 = singles.tile([128, 128], F32)
    make_identity(nc, id_f32)
    id_bf16 = singles.tile([128, 128], BF16)
    nc.gpsimd.tensor_copy(out=id_bf16, in_=id_f32)
    ones_col = singles.tile([1, 128], BF16)
    nc.vector.memset(ones_col, 1.0)
    eps_t = singles.tile([128, 1], F32)
    nc.vector.memset(eps_t, LN_EPS)
    absorb_scratch = singles.tile([128, 16], F32)
    absorb_n = [0]

    def absorb(ap):
        # DVE wait absorber: DVE-struct instructions support only one sync
        # wait on this codegen, so soak the DMA-completion wait into a copy.
        # Disjoint dest columns so absorbers carry no WAW dep on each other.
        k = absorb_n[0] % 16
        absorb_n[0] += 1
        nc.vector.tensor_copy(out=absorb_scratch[:, k:k + 1],
                              in_=ap[0:128, 0:1])
    v_aug = singles.tile([128, NCH, H, 34], BF16)
    nc.vector.memset(v_aug, 1.0)
    # xnT: [128 (4 heads x 32 dims), half, T] transposed LN'd activations
    xnT = singles.tile([128, 2, T], BF16)
    hT = singles.tile([128, 8, 512], BF16)

    def ln_stats(c, mv):
        st = sp.tile([128, 6], F32, tag="st")
        nc.vector.bn_stats(out=st, in_=x_sb[:, c, :])
        nc.vector.bn_aggr(out=mv[:, c, :], in_=st)

    def ln_rsqrt(mv, rs):
        nc.scalar.activation(
            out=rs, in_=mv[:, :, 1],
            func=mybir.ActivationFunctionType.Ln, bias=eps_t, scale=1.0,
        )
        nc.scalar.activation(
            out=rs, in_=rs,
            func=mybir.ActivationFunctionType.Exp, scale=-0.5,
        )

    def ln_norm(c, mv, rs, affine):
        """normalized bf16 chunk of x_sb[:, c, :]"""
        xn = xp.tile([128, D], BF16, tag="xn")
        if affine is not None:
            w_sb, b_sb = affine
            xf = xp.tile([128, D], F32, tag="xf")
            nc.vector.tensor_scalar(
                out=xf, in0=x_sb[:, c, :],
                scalar1=mv[:, c, 0:1], scalar2=rs[:, c:c + 1],
                op0=mybir.AluOpType.subtract, op1=mybir.AluOpType.mult,
            )
            nc.vector.tensor_mul(out=xf, in0=xf, in1=w_sb)
            nc.vector.tensor_add(out=xn, in0=xf, in1=b_sb)
        else:
            nc.vector.tensor_scalar(
                out=xn, in0=x_sb[:, c, :],
                scalar1=mv[:, c, 0:1], scalar2=rs[:, c:c + 1],
                op0=mybir.AluOpType.subtract, op1=mybir.AluOpType.mult,
            )
        return xn

    def transpose_to(xn, c):
        # one PE transpose per 128-col half into a single 1-bank PSUM tile,
        # then one DVE copy into both xnT halves.
        pt = ps_p.tile([128, 2, 128], BF16, tag="ps")
        for half in range(2):
            nc.tensor.transpose(
                pt[:, half, :], xn[:, half * 128:(half + 1) * 128], id_bf16
            )
        nc.vector.tensor_copy(
            out=xnT[:, :, c * 128:(c + 1) * 128], in_=pt
        )

    mvA = sp.tile([128, NCH, 2], F32, tag="mvA")
    for c in range(NCH):
        ln_stats(c, mvA)

    for l in range(L):
        # per-layer params
        ln1w_sb = lnp.tile([128, D], BF16, tag="ln1w")
        nc.sync.dma_start(out=ln1w_sb, in_=io["ln1w"][l])
        absorb(ln1w_sb)
        ln1b_sb = lnp.tile([128, D], BF16, tag="ln1b")
        nc.sync.dma_start(out=ln1b_sb, in_=io["ln1b"][l])
        absorb(ln1b_sb)
        w1_sb = wp.tile([128, D // 128, H2], BF16, tag="w1")
        nc.sync.dma_start(out=w1_sb, in_=io["w1h"][l])
        b1_sb = wp.tile([128, H2 // 128], F32, tag="b1")
        nc.sync.dma_start(out=b1_sb, in_=io["b1h"][l])
        w2_sb = wp.tile([128, H2 // 128, D], BF16, tag="w2")
        nc.sync.dma_start(out=w2_sb, in_=io["w2h"][l])
        b2_sb = wp.tile([1, D], BF16, tag="b2")
        nc.sync.dma_start(out=b2_sb, in_=io["b2h"][l])

        # ---- phase A: LN1 (stats in mvA from previous phase D) ----
        rsA = sp.tile([128, NCH], F32, tag="rsA")
        ln_rsqrt(mvA, rsA)
        for c in range(NCH):
            xn = ln_norm(c, mvA, rsA, (ln1w_sb, ln1b_sb))
            nc.gpsimd.tensor_copy(
                out=v_aug[:, c, :, 0:HD],
                in_=xn.rearrange("p (h d) -> p h d", h=H),
            )
            transpose_to(xn, c)

        # ---- phase B: attention, software-pipelined across head slots ----
        def qk_exp_ebm(b, h, jc):
            hh, hp = h // 4, (h % 4) * HD
            ps = ps_p.tile([128, S], F32, tag="ps")
            ktile = xnT[hp:hp + HD, hh,
                        b * S + jc * 128: b * S + (jc + 1) * 128]
            for it in range(2):
                qtile = xnT[hp:hp + HD, hh,
                            b * S + it * 512: b * S + (it + 1) * 512]
                nc.tensor.matmul(
                    ps[:, it * 512:(it + 1) * 512],
                    lhsT=ktile, rhs=qtile, start=True, stop=True,
                    tile_position=(hp, 0),
                )
            et = ep.tile([128, S], BF16, tag="et")
            nc.scalar.activation(
                out=et, in_=ps,
                func=mybir.ActivationFunctionType.Exp, scale=SCALE,
            )
            nc.vector.tensor_mul(out=et, in0=et, in1=eb_sb[:, h, jc, :])
            return et

        def pv(b, h, jc, po, etb):
            for it in range(2):
                nc.tensor.matmul(
                    po[:, it * 512:(it + 1) * 512],
                    lhsT=v_aug[:, b * 8 + jc, h, 0:33],
                    rhs=etb[:, it * 512:(it + 1) * 512],
                    start=(jc == 0), stop=(jc == S // 128 - 1),
                )

        def po_post(b, h, po):
            # denominator division + residual add, batched per (b, h)
            ot = otp.tile([33, S], BF16, tag="ot")
            nc.vector.tensor_copy(out=ot, in_=po)
            ptt = po_p.tile([128, 8, 34], BF16, tag="po")
            for ic in range(S // 128):
                nc.tensor.transpose(
                    ptt[:, ic, 0:33], ot[:, ic * 128:(ic + 1) * 128],
                    id_bf16[0:33, 0:33],
                )
            rt = sp.tile([128, 8, 1], F32, tag="rt")
            nc.vector.reciprocal(out=rt, in_=ptt[:, :, 32:33])
            tmp = tmp_p.tile([128, 8, HD], BF16, tag="tmp")
            nc.vector.tensor_tensor(
                out=tmp, in0=ptt[:, :, 0:HD],
                in1=rt[:, :, :].broadcast_to([128, 8, HD]),
                op=mybir.AluOpType.mult,
            )
            xs = x_sb[:, b * 8:(b + 1) * 8, h * HD:(h + 1) * HD]
            nc.vector.tensor_add(out=xs, in0=xs, in1=tmp)

        mvC = sp.tile([128, NCH, 2], F32, tag="mvC")
        pend = []
        po_cur = {}

        def drain_one():
            b0, h0, j0, e0 = pend.pop(0)
            if j0 == 0:
                po_t = po_p.tile([33, S], F32, tag="po")
                po_cur[(b0, h0)] = po_t
            pv(b0, h0, j0, po_cur[(b0, h0)], e0)
            if j0 == S // 128 - 1:
                po_post(b0, h0, po_cur.pop((b0, h0)))
                if h0 == H - 1:
                    # this batch's residual stream is final: LN2 stats can
                    # ride the DVE queue under the remaining attention slots
                    for c in range(b0 * 8, b0 * 8 + 8):
                        ln_stats(c, mvC)

        for b in range(BL):
            for h in range(H):
                for jc in range(S // 128):
                    etb = qk_exp_ebm(b, h, jc)
                    pend.append((b, h, jc, etb))
                    if len(pend) > 5:
                        drain_one()
        while pend:
            drain_one()

        # ---- phase C (LN2, affine folded into w1) fused with phase D ----
        rsC = sp.tile([128, NCH], F32, tag="rsC")
        ln_rsqrt(mvC, rsC)
        mvA = sp.tile([128, NCH, 2], F32, tag="mvA")
        for tt in range(T // 512):
            for c in range(4 * tt, 4 * tt + 4):
                transpose_to(ln_norm(c, mvC, rsC, None), c)
            for hbp in range(4):  # pairs of h2-blocks
                pm = ps_p.tile([128, S], F32, tag="ps")
                for sub in range(2):
                    hb = hbp * 2 + sub
                    for k in range(D // 128):
                        nc.tensor.matmul(
                            pm[:, sub * 512:(sub + 1) * 512],
                            lhsT=w1_sb[:, k, hb * 128:(hb + 1) * 128],
                            rhs=xnT[:, k, tt * 512:(tt + 1) * 512],
                            start=(k == 0), stop=(k == D // 128 - 1),
                        )
                for sub in range(2):
                    hb = hbp * 2 + sub
                    nc.scalar.activation(
                        out=hT[:, hb, :],
                        in_=pm[:, sub * 512:(sub + 1) * 512],
                        func=mybir.ActivationFunctionType.Gelu,
                        bias=b1_sb[:, hb:hb + 1],
                    )
            for t2 in range(4):
                pm2 = ps_p.tile([128, D], F32, tag="ps")
                for hb in range(H2 // 128):
                    nc.tensor.matmul(
                        pm2,
                        lhsT=hT[:, hb, t2 * 128:(t2 + 1) * 128],
                        rhs=w2_sb[:, hb, :],
                        start=(hb == 0), stop=False,
                    )
                nc.tensor.matmul(
                    pm2, lhsT=ones_col, rhs=b2_sb, start=False, stop=True
                )
                c = tt * 4 + t2
                nc.vector.tensor_add(
                    out=x_sb[:, c, :], in0=x_sb[:, c, :], in1=pm2
                )
                # stats for the next LN (LN1 of l+1, or the final LN)
                ln_stats(c, mvA)

    # ---- final LN (in place, f32) ----
    lnfw_sb = lnp.tile([128, D], BF16, tag="ln1w")
    nc.sync.dma_start(out=lnfw_sb, in_=io["lnfw"][:])
    absorb(lnfw_sb)
    lnfb_sb = lnp.tile([128, D], BF16, tag="ln1b")
    nc.sync.dma_start(out=lnfb_sb, in_=io["lnfb"][:])
    absorb(lnfb_sb)

    rsF = sp.tile([128, NCH], F32, tag="rsA")
    ln_rsqrt(mvA, rsF)
    for c in range(NCH):
        xc = x_sb[:, c, :]
        nc.vector.tensor_scalar(
            out=xc, in0=xc,
            scalar1=mvA[:, c, 0:1], scalar2=rsF[:, c:c + 1],
            op0=mybir.AluOpType.subtract, op1=mybir.AluOpType.mult,
        )
        nc.vector.tensor_mul(out=xc, in0=xc, in1=lnfw_sb)
        nc.vector.tensor_add(out=xc, in0=xc, in1=lnfb_sb)
    nc.sync.dma_start(out=y[:], in_=x_sb)


def _install_ntff_hook():
    """Wire antenv.axon_hooks NTFF profiling via libaxon ctypes (dev only)."""
    if _cache.get("hook_done"):
        return
    _cache["hook_done"] = True
    try:
        import types
        import sys
        try:
            from antenv.axon_hooks import set_axon_ntff_profile_hook  # noqa
        except ImportError:
            import antenv
            mod = types.ModuleType("antenv.axon_hooks")
            holder = [None]
            mod.set_axon_ntff_profile_hook = lambda h: holder.__setitem__(0, h)
            mod.get_axon_ntff_profile_hook = lambda: holder[0]
            sys.modules["antenv.axon_hooks"] = mod
            antenv.axon_hooks = mod
            from trn_agent_boot.trn_boot import _ntff_profile_via_ctypes
            mod.set_axon_ntff_profile_hook(
                _ntff_profile_via_ctypes("/opt/axon/libaxon_pjrt.so"))
    except Exception as e:  # fail-soft: tracing degrades, run still works
        print("ntff hook install failed:", e)


def kernel(tokens, pos_ids, emb_table, input_weight, position_weight,
           ln1_w, ln1_b, ln2_w, ln2_b, w1, b1, w2, b2, lnf_w, lnf_b):
    tokens = np.asarray(tokens)
    pos_ids = np.asarray(pos_ids)
    emb_table = np.asarray(emb_table, dtype=np.float32)
    x0 = (np.float32(np.asarray(input_weight).reshape(-1)[0])
          * emb_table[tokens]
          + np.float32(np.asarray(position_weight).reshape(-1)[0])
          * _pos_table()[np.asarray(pos_ids)][None]).astype(np.float32)

    w1 = np.asarray(w1, np.float32)
    b1 = np.asarray(b1, np.float32)
    w2 = np.asarray(w2, np.float32)
    b2 = np.asarray(b2, np.float32)
    ln2_w = np.asarray(ln2_w, np.float32)
    ln2_b = np.asarray(ln2_b, np.float32)
    # fold LN2 affine into MLP weights
    w1eff = ln2_w[:, :, None] * w1                     # [L, D, H2]
    b1eff = b1 + np.einsum("ld,ldh->lh", ln2_b, w1)    # [L, H2]
    w1h = np.ascontiguousarray(
        w1eff.reshape(L, D // 128, 128, H2).transpose(0, 2, 1, 3)
    ).astype(ml_dtypes.bfloat16)
    w2h = np.ascontiguousarray(
        w2.reshape(L, H2 // 128, 128, D).transpose(0, 2, 1, 3)
    ).astype(ml_dtypes.bfloat16)

    nc = _build_bass()
    base = {
        "ebT": _alibi_ebT(),
        "w1h": w1h,
        "b1h": np.ascontiguousarray(
            b1eff.reshape(L, H2 // 128, 128).transpose(0, 2, 1)),
        "w2h": w2h,
        "b2h": np.ascontiguousarray(
            np.asarray(b2)[:, None, :]).astype(ml_dtypes.bfloat16),
        "ln1w": np.ascontiguousarray(np.broadcast_to(
            np.asarray(ln1_w)[:, None, :], (L, 128, D))
        ).astype(ml_dtypes.bfloat16),
        "ln1b": np.ascontiguousarray(np.broadcast_to(
            np.asarray(ln1_b)[:, None, :], (L, 128, D))
        ).astype(ml_dtypes.bfloat16),
        "lnfw": np.ascontiguousarray(np.broadcast_to(
            np.asarray(lnf_w)[None, :], (128, D))
        ).astype(ml_dtypes.bfloat16),
        "lnfb": np.ascontiguousarray(np.broadcast_to(
            np.asarray(lnf_b)[None, :], (128, D))
        ).astype(ml_dtypes.bfloat16),
    }
    in_maps = []
    for core in range(NC):
        xc = x0[core * BL:(core + 1) * BL].reshape(T, D)
        xh = np.ascontiguousarray(
            xc.reshape(NCH, 128, D).transpose(1, 0, 2))
        m = dict(base)
        m["x0"] = xh
        in_maps.append(m)

    trace = os.environ.get("KERNEL_TRACE", "0") == "1"
    if trace:
        _install_ntff_hook()
    res = run_bass_kernel_spmd(
        nc, in_maps, core_ids=list(range(NC)), trace=trace,
        trace_cores=[0] if trace else None,
    )
    if trace and res.exec_time_ns is not None:
        print(f"HW exec time: {res.exec_time_ns} ns")
        if res.instructions_and_trace is not None:
            print("trace:", res.instructions_and_trace[1])

    out = np.empty((B, S, D), np.float32)
    for core in range(NC):
        yh = res.results[core]["y"]  # [128, NCH, D]
        yc = yh.transpose(1, 0, 2).reshape(BL, S, D)
        out[core * BL:(core + 1) * BL] = yc
    return out
